# revision 5
# baseline (speedup 1.0000x reference)
"""Bahdanau attention kernel for Trainium2 (8 NeuronCores, data-parallel over batch).

reference:
    cat    = concat([enc, tile(hidden)], -1)          # [B, S, H]
    energy = tanh(cat @ W_attn.T + b_attn)            # [B, S, H]
    scores = energy @ v_w                             # [B, S]
    attn   = softmax(scores, axis=1)
    ctx    = attn @ enc                               # [B, D]

Host-side algebra: split W_attn = [W1 | W2] along its input dim. The W2 (hidden)
half collapses to a per-batch bias hb = hidden @ W2.T + b_attn, so on-device
work is just  tanh(enc @ W1.T + hb) -> v-dot -> softmax -> weighted sum.
Each core handles B/8 = 2 batches; no collectives.

Device layout per 512-seq tile (per batch):
  energy MM : out [o_chunk=128, s=512] psum, K=512 (4 accum steps),
              lhsT = W1T chunk [128h, 128o], rhs = encT tile [128h, 512s]
  tanh      : ScalarE activation, per-partition bias hb[o] (free), psum -> sbuf bf16
  scores    : v-stationary MMs, lhsT = v chunk [128o, 1], rhs = energy [128o, 512s],
              accumulated over 8 o-chunks -> psum [1, 512]
  exp       : ScalarE Exp (scores are bounded |s| <= ||v||_1 ~ 25, so no max
              subtraction needed; softmax is shift-invariant), accum_out gives
              the per-tile partial normalizer Z for free
  p^T       : 4 K=1 matmuls (lhsT = p[1,128] chunk, rhs = ones[1,1]) -> [128,4]
  context   : p-stationary MMs, lhsT = pT[:, i] [128s, 1], rhs = enc tile
              [128s, 512d] -> psum [1, 512] accumulated over the whole batch
  finish    : ctx * (1/Z) on VectorE, DMA out.
"""

import numpy as np
import ml_dtypes

import concourse.bacc as bacc
import concourse.bass as bass
import concourse.tile as tile
from concourse import mybir
from concourse.bass_utils import run_bass_kernel_spmd

B, S, D, H = 16, 8192, 512, 1024
NCORES = 8
BPC = B // NCORES          # batches per core
ST = 512                   # seq tile size
NT = S // ST               # seq tiles per batch
KC = D // 128              # 4 contraction chunks for energy MM
OC = H // 128              # 8 o-chunks
NSUB = ST // 128           # 4 128-sub-tiles inside a seq tile

F32 = mybir.dt.float32
BF16 = mybir.dt.bfloat16
BF = ml_dtypes.bfloat16
AX = mybir.AxisListType.X
TANH = mybir.ActivationFunctionType.Tanh
EXP = mybir.ActivationFunctionType.Exp

_CACHE = {}


def build_nc():
    nc = bacc.Bacc(None, target_bir_lowering=False)
    encT = nc.declare_dram_parameter("encT", [BPC, D, S], BF16, isOutput=False)
    encN = nc.declare_dram_parameter("encN", [BPC, S, D], BF16, isOutput=False)
    w1t = nc.declare_dram_parameter("w1t", [D, H], BF16, isOutput=False)
    hb = nc.declare_dram_parameter("hb", [BPC, H], F32, isOutput=False)
    vw = nc.declare_dram_parameter("vw", [H], BF16, isOutput=False)
    out = nc.declare_dram_parameter("out", [BPC, D], F32, isOutput=True)

    with tile.TileContext(nc) as tc:
        with (
            tc.tile_pool(name="singles", bufs=1) as singles,
            tc.tile_pool(name="enc", bufs=3) as encp,
            tc.tile_pool(name="energy", bufs=3) as enp,
            tc.tile_pool(name="small", bufs=4) as smallp,
            tc.tile_pool(name="batch", bufs=2) as batchp,
            tc.tile_pool(name="eps", bufs=2, space="PSUM") as eps_pool,
            tc.tile_pool(name="scps", bufs=2, space="PSUM") as scps_pool,
            tc.tile_pool(name="ptps", bufs=2, space="PSUM") as ptps_pool,
            tc.tile_pool(name="ctxps", bufs=2, space="PSUM") as ctxps_pool,
        ):
            # ---- setup (once) ----
            w1t_sb = singles.tile([128, KC, H], BF16)
            nc.sync.dma_start(
                out=w1t_sb, in_=w1t.rearrange("(kc p) o -> p kc o", p=128)
            )
            v_sb = singles.tile([128, OC], BF16)
            nc.gpsimd.dma_start(out=v_sb, in_=vw.rearrange("(oc p) -> p oc", p=128))
            hb_sb = singles.tile([128, BPC, OC], F32)
            nc.gpsimd.dma_start(
                out=hb_sb, in_=hb.rearrange("b (oc p) -> p b oc", p=128)
            )
            ones_sb = singles.tile([1, 1], BF16)
            nc.vector.memset(ones_sb, 1.0)

            for b in range(BPC):
                ctx_ps = ctxps_pool.tile([1, D], F32)
                zbuf = batchp.tile([1, NT], F32)
                for t in range(NT):
                    # ---- loads ----
                    encT_sb = encp.tile([128, KC, ST], BF16)
                    nc.sync.dma_start(
                        out=encT_sb,
                        in_=encT[b, :, t * ST:(t + 1) * ST].rearrange(
                            "(kc p) s -> p kc s", p=128
                        ),
                    )
                    encN_sb = encp.tile([128, NSUB, D], BF16)
                    nc.sync.dma_start(
                        out=encN_sb,
                        in_=encN[b, t * ST:(t + 1) * ST, :].rearrange(
                            "(i p) d -> p i d", p=128
                        ),
                    )
                    # ---- energy + scores ----
                    sc_ps = scps_pool.tile([1, ST], F32)
                    for oc in range(OC):
                        e_ps = eps_pool.tile([128, ST], F32)
                        for kc in range(KC):
                            nc.tensor.matmul(
                                e_ps,
                                lhsT=w1t_sb[:, kc, oc * 128:(oc + 1) * 128],
                                rhs=encT_sb[:, kc, :],
                                start=(kc == 0),
                                stop=(kc == KC - 1),
                            )
                        e_sb = enp.tile([128, ST], BF16)
                        nc.scalar.activation(
                            e_sb, e_ps, TANH, bias=hb_sb[:, b, oc:oc + 1]
                        )
                        nc.tensor.matmul(
                            sc_ps,
                            lhsT=v_sb[:, oc:oc + 1],
                            rhs=e_sb,
                            start=(oc == 0),
                            stop=(oc == OC - 1),
                        )
                    # ---- exp (+ partial Z via accum_out) ----
                    p_sb = smallp.tile([1, ST], BF16)
                    nc.scalar.activation(
                        p_sb, sc_ps, EXP, accum_out=zbuf[:, t:t + 1]
                    )
                    # ---- transpose p to [128, NSUB] ----
                    pt_ps = ptps_pool.tile([128, NSUB], F32)
                    for i in range(NSUB):
                        nc.tensor.matmul(
                            pt_ps[:, i:i + 1],
                            lhsT=p_sb[:, i * 128:(i + 1) * 128],
                            rhs=ones_sb,
                            start=True,
                            stop=True,
                        )
                    pt_sb = smallp.tile([128, NSUB], BF16)
                    nc.vector.tensor_copy(pt_sb, pt_ps)
                    # ---- context accumulation ----
                    for i in range(NSUB):
                        nc.tensor.matmul(
                            ctx_ps,
                            lhsT=pt_sb[:, i:i + 1],
                            rhs=encN_sb[:, i, :],
                            start=(t == 0 and i == 0),
                            stop=(t == NT - 1 and i == NSUB - 1),
                        )
                # ---- normalize + store ----
                zsum = smallp.tile([1, 1], F32)
                nc.vector.reduce_sum(out=zsum, in_=zbuf, axis=AX)
                rz = smallp.tile([1, 1], F32)
                nc.vector.reciprocal(rz, zsum)
                ctx_sb = smallp.tile([1, D], F32)
                nc.vector.tensor_scalar_mul(ctx_sb, ctx_ps, rz)
                nc.sync.dma_start(out=out[b:b + 1, :], in_=ctx_sb)
    nc.compile()
    return nc


def _prep_inputs(encoder_outputs, hidden, W_attn, b_attn, v_w):
    enc_bf = encoder_outputs.astype(BF)
    encT_bf = np.ascontiguousarray(enc_bf.transpose(0, 2, 1))
    hb = (hidden.astype(np.float64) @ W_attn[:, D:].T.astype(np.float64)
          + b_attn.astype(np.float64)).astype(np.float32)
    w1t_bf = np.ascontiguousarray(W_attn[:, :D].T).astype(BF)
    v_bf = v_w.astype(BF)
    in_maps = []
    for c in range(NCORES):
        sl = slice(c * BPC, (c + 1) * BPC)
        in_maps.append({
            "encT": encT_bf[sl],
            "encN": enc_bf[sl],
            "w1t": w1t_bf,
            "hb": hb[sl],
            "vw": v_bf,
        })
    return in_maps


def _run(in_maps, trace=False):
    if "nc" not in _CACHE:
        _CACHE["nc"] = build_nc()
    nc = _CACHE["nc"]
    kw = {}
    if trace:
        import os
        os.makedirs("/tmp/bass_trace", exist_ok=True)
        kw = {"tmpdir": "/tmp/bass_trace"}
    res = run_bass_kernel_spmd(nc, in_maps, list(range(NCORES)), trace=trace, **kw)
    out = np.concatenate([res.results[c]["out"] for c in range(NCORES)], axis=0)
    return out.astype(np.float32), res


def kernel(**inputs):
    in_maps = _prep_inputs(
        inputs["encoder_outputs"], inputs["hidden"], inputs["W_attn"],
        inputs["b_attn"], inputs["v_w"],
    )
    out, _ = _run(in_maps, trace=False)
    return out


def kernel_traced(**inputs):
    """test.py entry: also returns BassKernelResults with profile info."""
    in_maps = _prep_inputs(
        inputs["encoder_outputs"], inputs["hidden"], inputs["W_attn"],
        inputs["b_attn"], inputs["v_w"],
    )
    return _run(in_maps, trace=True)
